# revision 31
# baseline (speedup 1.0000x reference)
"""Jacobi 100-step solver on 8 trn2 cores via truncated DST-spectral transform.

v4: one explicit Jacobi step folded into mode space. With Qc the [N, K]
interior-DST basis (zero at boundary rows), the masked stencil satisfies
G1 = Qc^T x1 Qc = s_ab * G0 + 0.25*(rank-1 boundary spikes), G0 = Qc^T x0 Qc,
s_ab = lam_a + lam_b. The s_ab multiply rides the matmuls:
s_ab*G0 = (Qrows lam)^T B + Qrows^T (B lam), so the spikes share one PSUM
accumulation group and the G drain is a plain copy. x100 = Qc [W99*G1] Qc^T
on the (lo,lo)+(hi,hi) corners (K=256). Backward halves via Qhi = D Qlo:
out_odd = QloT_odd^T (Zlo+Zhi), out_even with (Zlo-Zhi), QloT parity-packed
on host. Forward hi-corner uses x0d = D*x0 against the same qlo panel.

Schedule notes (from neuron-profile traces): the 8 SPMD cores launch with
~30us skew and the first collective pays a barrier + setup latency, so a
zero-payload warmup AllReduce is doorbelled at ~11us (barrier_in DMA rides
first on the sync ring); the real AllReduce then chains ~2us after its
doorbell. All loads are single large DMA triggers striped over all 16 DMA
engines, ordered so forward-critical bytes land first; xy is quartered so
elementwise block 0 starts ~5us earlier; backward consts stream during the
AllReduce window. Spike vectors upload as a [128,16] compact blob and are
scattered to a [1, 2048] row by a tiny SBUF-to-SBUF DMA.
"""

import sys
import types
import numpy as np

N = 2048
NC = 8
P = N // NC          # 256 panel columns per core
K = 256              # spectral corner size per corner
RC = N // 128        # 16 row chunks

# qb blob (shared across cores): [qloA 2048 | ident 128 | lamr 1024 | qloB 2048]
QB_IDENT = 2048
QB_LAMR = 2176       # (jm, t) slot s=2jm+t -> lam_t[a] replicated over partitions
QB_B = 3200
QBW = 5248
# cbQ blob (per-core): [qrows 1024 | qrowsL 1024 | svc 16]
OF_QROWS = 0         # (jm, t) -> Qc_t[256*core + jm*128 + p, b]
OF_QROWSL = 1024     # same * lam_t[b]
OF_SVC = 2048        # [128, 16] compact: (p, j) = svflat[p*16 + j]
CBQW = 2064
# svflat row (scattered to svusv[0:1, 0:2048]):
# [0:1024]   svu: svec_lo | svecp_lo | svec_hi | svecp_hi       (x0.25)
# [1024:2048] svv: vleft lo | vleft hi | vright lo | vright hi  (x0.25, edge sel)
# cbB blob (backward consts): [qrowsT 1024 | w99 1024]
OF_QROWST = 0        # (t, bm) -> Qc_t[256*core + c, bm*128 + p]
OF_W99 = 1024        # (t, bm) -> W99_t[bm*128+p, a]
CBBW = 2048


def _install_ntff_hook():
    if "antenv.axon_hooks" in sys.modules:
        return
    mod = types.ModuleType("antenv.axon_hooks")
    mod._hook = None
    mod.set_axon_ntff_profile_hook = lambda h: setattr(mod, "_hook", h)
    mod.get_axon_ntff_profile_hook = lambda: mod._hook
    sys.modules["antenv.axon_hooks"] = mod
    try:
        import antenv
        antenv.axon_hooks = mod
        from trn_agent_boot.trn_boot import _ntff_profile_via_ctypes
        h = _ntff_profile_via_ctypes("/opt/axon/libaxon_pjrt.so")
        if h is not None:
            mod.set_axon_ntff_profile_hook(h)
    except Exception:
        pass


_HOST_CACHE = {}


def _host_constants():
    if _HOST_CACHE:
        return _HOST_CACHE
    i = np.arange(N, dtype=np.float64)
    Qs, lams, svecs, svecps = [], [], [], []
    for lo in (True, False):
        m = np.arange(1, K + 1, dtype=np.float64) if lo else np.arange(N - 2, N - 2 - K, -1, dtype=np.float64)
        red = np.outer(i, m) % (2 * (N - 1))
        Qc = np.sqrt(2.0 / (N - 1)) * np.sin(np.pi * red / (N - 1))   # [N, K]
        lam = 0.5 * np.cos(np.pi * m / (N - 1))
        Qs.append(Qc)
        lams.append(lam)
        svecs.append(Qc[1, :].copy())
        svecps.append(Qc[N - 2, :].copy())

    qb = np.zeros((128, QBW), np.float64)
    for r in range(RC):
        off = r * 256 if r < 8 else QB_B + (r - 8) * 256
        qb[:, off: off + 256] = Qs[0][128 * r:128 * (r + 1), :]
    qb[:, QB_IDENT: QB_IDENT + 128] = np.eye(128)
    for jm in range(2):
        for t in range(2):
            s = 2 * jm + t
            qb[:, QB_LAMR + s * 256: QB_LAMR + (s + 1) * 256] = lams[t][None, :]

    # qcT parity packed [128, 4096]: [am, par, ic, j] = Qlo[256*ic + 2*j + par, 128*am + p]
    qcT = np.zeros((128, 4096), np.float64)
    j = np.arange(128)
    for am in range(2):
        for par in range(2):
            for ic in range(8):
                rows = 256 * ic + 2 * j + par
                qcT[:, am * 2048 + par * 1024 + ic * 128: am * 2048 + par * 1024 + (ic + 1) * 128] = \
                    Qs[0][rows, 128 * am:128 * (am + 1)].T

    w99T = np.zeros((128, 1024), np.float64)
    for t in range(2):
        sab = lams[t][:, None] + lams[t][None, :]      # [b, a] (symmetric)
        w99 = sab ** 99
        for bm in range(2):
            s = 2 * t + bm
            w99T[:, s * 256:(s + 1) * 256] = w99[bm * 128:(bm + 1) * 128, :]

    _HOST_CACHE.update(qb=qb.astype(np.float16), qcT=qcT.astype(np.float16),
                       w99T=w99T, lams=lams, Qs=Qs, svecs=svecs, svecps=svecps)
    return _HOST_CACHE


def _core_cbQ(c, hc):
    Qs, svecs, svecps, lams = hc["Qs"], hc["svecs"], hc["svecps"], hc["lams"]
    cb = np.zeros((128, CBQW), np.float64)
    for jm in range(2):
        for t in range(2):
            s = 2 * jm + t
            rows = Qs[t][256 * c + 128 * jm: 256 * c + 128 * (jm + 1), :]
            cb[:, OF_QROWS + s * 256: OF_QROWS + (s + 1) * 256] = rows
            cb[:, OF_QROWSL + s * 256: OF_QROWSL + (s + 1) * 256] = rows * lams[t][None, :]
    svflat = np.zeros(2048, np.float64)
    for t in range(2):
        svflat[t * 512: t * 512 + 256] = 0.25 * svecs[t]
        svflat[t * 512 + 256: t * 512 + 512] = 0.25 * svecps[t]
    if c == 0:
        for t in range(2):
            svflat[1024 + t * 256: 1024 + (t + 1) * 256] = 0.25 * svecs[t]
    if c == NC - 1:
        for t in range(2):
            svflat[1536 + t * 256: 1536 + (t + 1) * 256] = 0.25 * svecps[t]
    cb[:, OF_SVC: OF_SVC + 16] = svflat.reshape(128, 16)
    return cb.astype(np.float16)


def _core_cbB(c, hc):
    Qs = hc["Qs"]
    cb = np.zeros((128, CBBW), np.float64)
    for t in range(2):
        for bm in range(2):
            s = 2 * t + bm
            cb[:, OF_QROWST + s * 256: OF_QROWST + (s + 1) * 256] = \
                Qs[t][256 * c: 256 * (c + 1), 128 * bm: 128 * (bm + 1)].T
    cb[:, OF_W99: OF_W99 + 1024] = hc["w99T"]
    return cb.astype(np.float16)


_NC_CACHE = {}


def _build():
    if "nc" in _NC_CACHE:
        return _NC_CACHE["nc"]
    import concourse.bacc as bacc
    import concourse.tile as tile
    import concourse.mybir as mybir

    F16 = mybir.dt.float16
    F32 = mybir.dt.float32
    ALU = mybir.AluOpType
    ACTF = mybir.ActivationFunctionType
    nc = bacc.Bacc("TRN2", target_bir_lowering=False, debug=False, num_devices=NC)

    # xyb col 0 = sgncol ((-1)^(p+1)); then 4 blocks of [x 1024 | y 1024]
    xyb_d = nc.dram_tensor("xyb", [128, 8193], F16, kind="ExternalInput")
    qb_d = nc.dram_tensor("qb", [128, QBW], F16, kind="ExternalInput")
    cbQ_d = nc.dram_tensor("cbQ", [128, CBQW], F16, kind="ExternalInput")
    cbB_d = nc.dram_tensor("cbB", [128, CBBW], F16, kind="ExternalInput")
    qcT_d = nc.dram_tensor("qcT", [128, 4096], F16, kind="ExternalInput")
    out_d = nc.dram_tensor("out", [128, 4096], F16, kind="ExternalOutput")

    with tile.TileContext(nc) as tc:
        with tc.tile_pool(name="pers", bufs=1) as pers, \
             tc.tile_pool(name="ps", bufs=1, space="PSUM") as ps, \
             tc.tile_pool(name="dram", bufs=1, space="DRAM") as dram:

            # ---- persistent SBUF ----
            xyb_s = pers.tile([128, 8193], F16, tag="xyb")
            qb_s = pers.tile([128, QBW], F16, tag="qb")
            cbQ_s = pers.tile([128, CBQW], F16, tag="cbQ")
            cbB_s = pers.tile([128, CBBW], F16, tag="cbB")
            qcT_s = pers.tile([128, 4096], F16, tag="qcT")
            x0b = pers.tile([128, 4096], F16, tag="x0b")
            x0d = pers.tile([128, 4096], F16, tag="x0d")
            t2b = pers.tile([128, 4096], F32, tag="t2b")
            t3b = pers.tile([128, 4096], F32, tag="t3b")
            d2b = pers.tile([128, 4096], F32, tag="d2b")
            abuf = pers.tile([128, 1024], F16, tag="abuf")
            abufL = pers.tile([128, 1024], F16, tag="abufL")
            usb = pers.tile([128, 1024], F16, tag="usb")
            svusv = pers.tile([128, 2048], F16, tag="svusv")
            x0rT = pers.tile([128, 4], F16, tag="x0rT")
            vrow = pers.tile([128, 512], F16, tag="vrow")
            gsb = pers.tile([128, 1024], F16, tag="gsb")
            gout_s = pers.tile([128, 1024], F16, tag="gouts")
            utb = pers.tile([128, 1024], F16, tag="utb")
            zbuf = pers.tile([128, 1024], F16, tag="zbuf")
            ztmp = pers.tile([128, 512], F16, tag="ztmp")
            outb = pers.tile([128, 4096], F16, tag="outb")
            zt = pers.tile([128, 1], F32, tag="zt")

            # const APs for activation biases
            cexp = pers.tile([128, 1], F32, tag="cexp", name="cexp")
            nc.vector.memset(cexp[:], -12.5)
            nc.const_aps.aps[(F32, -12.5)] = cexp[:]
            csq = pers.tile([128, 1], F32, tag="csq", name="csq")
            nc.vector.memset(csq[:], -0.5)
            nc.const_aps.aps[(F32, -0.5)] = csq[:]

            # ---- warmup AllReduce: barrier_in rides FIRST on the sync ring so
            # the gpsimd doorbell rings at ~11us and the NEFF collective
            # barrier + first-op setup burn during forward ----
            barrier_in = dram.tile([128, 1], F32, tag="barin")
            barrier_out = dram.tile([128, 1], F32, tag="barout", addr_space="Shared")
            nc.vector.memset(zt[:], 0.0)
            nc.sync.dma_start(barrier_in[:, :], zt[:])
            nc.gpsimd.collective_compute(
                "AllReduce", ALU.add,
                replica_groups=[list(range(NC))],
                ins=[barrier_in.opt()], outs=[barrier_out.opt()],
            )

            # ---- loads: strict FIFO order on the sync ring; xy quartered ----
            nc.sync.dma_start(xyb_s[:, 0:2049], xyb_d[:, 0:2049])        # sgn + blk0
            nc.sync.dma_start(xyb_s[:, 2049:4097], xyb_d[:, 2049:4097])  # blk1
            nc.sync.dma_start(qb_s[:, 0:QB_B], qb_d[:, 0:QB_B])          # qloA+ident+lamr
            nc.sync.dma_start(xyb_s[:, 4097:6145], xyb_d[:, 4097:6145])  # blk2
            nc.sync.dma_start(xyb_s[:, 6145:8193], xyb_d[:, 6145:8193])  # blk3
            nc.sync.dma_start(qb_s[:, QB_B:QBW], qb_d[:, QB_B:QBW])      # qloB
            nc.sync.dma_start(cbQ_s[:], cbQ_d[:, :])
            nc.sync.dma_start(cbB_s[:], cbB_d[:, :])
            nc.sync.dma_start(qcT_s[:], qcT_d[:, :])
            # scatter compact spike vectors to a [1, 2048] row (partition 0);
            # on the sync ring so its wait-for-cbQ doesn't stall the scalar queue
            nc.sync.dma_start(svusv[0:1, 0:2048], cbQ_s[:, OF_SVC:OF_SVC + 16])

            sgncol = xyb_s[:, 0:1]
            ident_s = qb_s[:, QB_IDENT:QB_IDENT + 128]

            # ---- A accumulators: [c-part(jm), modes] x (lo, hi) ----
            aps = [ps.tile([128, K], F32, tag="aacc", bufs=4, name=f"aps{j}") for j in range(4)]
            # slot j = 2*jm + t

            def qslot(r):
                off = r * 256 if r < 8 else QB_B + (r - 8) * 256
                return qb_s[:, off: off + 256]

            def mm1_chunk(r):
                for jm in range(2):
                    sl = slice(r * 256 + jm * 128, r * 256 + (jm + 1) * 128)
                    nc.tensor.matmul(aps[2 * jm][:], x0b[:, sl], qslot(r),
                                     start=(r == 0), stop=(r == RC - 1))
                for jm in range(2):
                    sl = slice(r * 256 + jm * 128, r * 256 + (jm + 1) * 128)
                    nc.tensor.matmul(aps[2 * jm + 1][:], x0d[:, sl], qslot(r),
                                     start=(r == 0), stop=(r == RC - 1))

            def fwd_block(blk):
                sl = slice(blk * 1024, (blk + 1) * 1024)
                xs = xyb_s[:, 1 + blk * 2048: 1 + blk * 2048 + 1024]
                ys = xyb_s[:, 1 + blk * 2048 + 1024: 1 + (blk + 1) * 2048]
                # d2' = X(X-1) + (Y-.5)^2 = d^2 - 0.25
                nc.vector.scalar_tensor_tensor(t2b[:, sl], xs, -1.0, xs, ALU.add, ALU.mult)
                nc.scalar.activation(t3b[:, sl], ys, ACTF.Square, bias=-0.5, scale=1.0)
                nc.gpsimd.tensor_add(d2b[:, sl], t2b[:, sl], t3b[:, sl])
                # x0 = exp(-50*d^2) = exp(-50*d2' - 12.5)
                nc.scalar.activation(x0b[:, sl], d2b[:, sl], ACTF.Exp, bias=-12.5, scale=-50.0)
                # hi-corner operand: x0d = D x0 (checkerboard row sign)
                nc.vector.tensor_mul(x0d[:, sl], x0b[:, sl], sgncol.to_broadcast((128, 1024)))
                for r in range(4 * blk, 4 * blk + 4):
                    mm1_chunk(r)

            def row_transpose(col_lo, out_col, take_row):
                tp = ps.tile([128, 128], F16, tag="pp", bufs=4, name="pp")
                nc.tensor.transpose(tp[:], x0b[:, col_lo:col_lo + 128], ident_s)
                nc.vector.tensor_copy(x0rT[:, out_col:out_col + 1], tp[:, take_row:take_row + 1])

            def u_project(xcol0, xcol1, dst_off):
                for t in range(2):
                    ups = ps.tile([128, K], F32, tag="pp", bufs=4, name="pp")
                    nc.tensor.matmul(ups[0:1, :], x0rT[:, xcol0:xcol0 + 1],
                                     cbQ_s[:, OF_QROWS + t * 256: OF_QROWS + (t + 1) * 256],
                                     start=True, stop=False)
                    nc.tensor.matmul(ups[0:1, :], x0rT[:, xcol1:xcol1 + 1],
                                     cbQ_s[:, OF_QROWS + (2 + t) * 256: OF_QROWS + (3 + t) * 256],
                                     start=False, stop=True)
                    nc.vector.tensor_copy(usb[0:1, dst_off + t * 256: dst_off + (t + 1) * 256], ups[0:1, :])

            fwd_block(0)
            fwd_block(1)
            # top boundary row machinery early (ident lands with qbA)
            row_transpose(0, 0, 0)
            row_transpose(128, 1, 0)
            fwd_block(2)
            fwd_block(3)
            row_transpose(15 * 256, 2, 127)
            row_transpose(15 * 256 + 128, 3, 127)
            u_project(0, 1, 0)                     # usb[0:512] = u_top (lo|hi)
            u_project(2, 3, 512)                   # usb[512:1024] = u_bot (lo|hi)

            # ---- drain A -> abuf and abufL = A * lam_a ----
            for j in range(4):
                jm, t = j // 2, j % 2
                sl = slice(jm * 512 + t * 256, jm * 512 + (t + 1) * 256)
                nc.scalar.copy(abuf[:, sl], aps[j][:])
                nc.vector.tensor_mul(abufL[:, sl], aps[j][:], qb_s[:, QB_LAMR + j * 256: QB_LAMR + (j + 1) * 256])

            # v_right row (panel col 255) lives at abuf partition 127; PE operands
            # must start at partition 0 -> stage it down via SBUF-to-SBUF DMA
            nc.sync.dma_start(vrow[0:1, :], abuf[127:128, 512:1024])

            # ---- mm2: G1^T = (Qrows L)^T B + Qrows^T (B L) + rank-1 boundary
            # spikes, one psum accumulation group per (t, bm) ----
            gin = dram.tile([128, 1024], F16, tag="gin")
            gout = dram.tile([128, 1024], F16, tag="gout", addr_space="Shared")
            for t in range(2):
                for bm in range(2):
                    s = 2 * t + bm
                    gp = ps.tile([128, K], F32, tag="pp", bufs=4, name="pp")
                    for jm in range(2):
                        qsl = slice(OF_QROWS + (2 * jm + t) * 256 + bm * 128,
                                    OF_QROWS + (2 * jm + t) * 256 + (bm + 1) * 128)
                        qLsl = slice(OF_QROWSL + (2 * jm + t) * 256 + bm * 128,
                                     OF_QROWSL + (2 * jm + t) * 256 + (bm + 1) * 128)
                        asl = slice(jm * 512 + t * 256, jm * 512 + (t + 1) * 256)
                        nc.tensor.matmul(gp[:], cbQ_s[:, qLsl], abuf[:, asl],
                                         start=(jm == 0), stop=False)
                        nc.tensor.matmul(gp[:], cbQ_s[:, qsl], abufL[:, asl],
                                         start=False, stop=False)
                    nc.tensor.matmul(gp[:], usb[0:1, t * 256 + bm * 128: t * 256 + (bm + 1) * 128],
                                     svusv[0:1, t * 512: t * 512 + 256],
                                     start=False, stop=False)
                    nc.tensor.matmul(gp[:], usb[0:1, 512 + t * 256 + bm * 128: 512 + t * 256 + (bm + 1) * 128],
                                     svusv[0:1, t * 512 + 256: t * 512 + 512],
                                     start=False, stop=False)
                    nc.tensor.matmul(gp[:], svusv[0:1, 1024 + t * 256 + bm * 128: 1024 + t * 256 + (bm + 1) * 128],
                                     abuf[0:1, t * 256: (t + 1) * 256],
                                     start=False, stop=False)
                    nc.tensor.matmul(gp[:], svusv[0:1, 1536 + t * 256 + bm * 128: 1536 + t * 256 + (bm + 1) * 128],
                                     vrow[0:1, t * 256: (t + 1) * 256],
                                     start=False, stop=True)
                    if s % 2 == 0:
                        nc.vector.tensor_copy(gsb[:, s * 256:(s + 1) * 256], gp[:])
                    else:
                        nc.scalar.copy(gsb[:, s * 256:(s + 1) * 256], gp[:])
            nc.sync.dma_start(gin[:, :], gsb[:])
            nc.gpsimd.collective_compute(
                "AllReduce", ALU.add,
                replica_groups=[list(range(NC))],
                ins=[gin.opt()], outs=[gout.opt()],
            )
            nc.sync.dma_start(gout_s[:], gout[:, :])

            # ---- filter: U^T = W99 * G1^T (SBUF-only) ----
            engs = (nc.vector, nc.gpsimd)
            for s in range(4):
                sl = slice(s * 256, (s + 1) * 256)
                engs[s % 2].tensor_mul(utb[:, sl], gout_s[:, sl],
                                       cbB_s[:, OF_W99 + s * 256: OF_W99 + (s + 1) * 256])

            # ---- B1: Z_t[a, c] = sum_b U_t[b, a] Qrows_t[c, b] ----
            # Zp (odd rows) at zbuf[0:512], Zm (even rows) at zbuf[512:1024]
            for am in range(2):
                zps = []
                for t in range(2):
                    zp = ps.tile([128, K], F32, tag="pp", bufs=4, name="pp")
                    for bm in range(2):
                        s = 2 * t + bm
                        nc.tensor.matmul(zp[:],
                                         utb[:, s * 256 + am * 128: s * 256 + (am + 1) * 128],
                                         cbB_s[:, OF_QROWST + s * 256: OF_QROWST + (s + 1) * 256],
                                         start=(bm == 0), stop=(bm == 1))
                    zps.append(zp)
                zsl = slice(am * 256, (am + 1) * 256)
                nc.scalar.copy(ztmp[:, zsl], zps[0][:])
                nc.vector.tensor_add(zbuf[:, am * 256: (am + 1) * 256], ztmp[:, zsl], zps[1][:])
                nc.vector.tensor_sub(zbuf[:, 512 + am * 256: 512 + (am + 1) * 256], ztmp[:, zsl], zps[1][:])

            # ---- B2: out chunks via parity-packed QloT; staged output DMA ----
            dr_engs = (nc.scalar, nc.vector)
            di = 0
            for ic in range(8):
                for par in range(2):
                    ops = ps.tile([128, K], F32, tag="pp", bufs=4, name="pp")
                    zoff = 0 if par == 1 else 512
                    for am in range(2):
                        nc.tensor.matmul(ops[:],
                                         qcT_s[:, am * 2048 + par * 1024 + ic * 128: am * 2048 + par * 1024 + (ic + 1) * 128],
                                         zbuf[:, zoff + am * 256: zoff + (am + 1) * 256],
                                         start=(am == 0), stop=(am == 1))
                    dst = outb[:, (2 * ic + par) * 256: (2 * ic + par + 1) * 256]
                    eng = dr_engs[di % 2]; di += 1
                    if eng is nc.scalar:
                        eng.copy(dst, ops[:])
                    else:
                        eng.tensor_copy(dst, ops[:])
                if ic == 3:
                    nc.scalar.dma_start(out_d[:, 0:2048], outb[:, 0:2048])
                elif ic == 5:
                    nc.scalar.dma_start(out_d[:, 2048:3072], outb[:, 2048:3072])
            nc.scalar.dma_start(out_d[:, 3072:4096], outb[:, 3072:4096])

    nc.compile()
    _NC_CACHE["nc"] = nc
    return nc


def _run(X, Y, trace=False):
    _install_ntff_hook()
    from concourse.bass_utils import run_bass_kernel_spmd

    hc = _host_constants()
    Xf = np.asarray(X, np.float32).astype(np.float16)
    Yf = np.asarray(Y, np.float32).astype(np.float16)
    sgn = np.where(np.arange(128) % 2 == 1, 1.0, -1.0).astype(np.float16)

    in_maps = []
    for c in range(NC):
        xp = Xf[:, P * c: P * (c + 1)]           # [2048, 256]
        yp = Yf[:, P * c: P * (c + 1)]
        xyb = np.zeros((128, 8193), np.float16)
        xyb[:, 0] = sgn
        for r in range(RC):
            b, rr = r // 4, r % 4
            xyb[:, 1 + b * 2048 + rr * 256: 1 + b * 2048 + (rr + 1) * 256] = xp[128 * r:128 * (r + 1), :]
            xyb[:, 1 + b * 2048 + 1024 + rr * 256: 1 + b * 2048 + 1024 + (rr + 1) * 256] = yp[128 * r:128 * (r + 1), :]
        m = {"xyb": xyb,
             "qb": hc["qb"],
             "cbQ": _core_cbQ(c, hc),
             "cbB": _core_cbB(c, hc),
             "qcT": hc["qcT"]}
        in_maps.append(m)

    nc = _build()
    r = run_bass_kernel_spmd(nc, in_maps, core_ids=list(range(NC)), trace=trace)
    panels = []
    for c in range(NC):
        o = r.results[c]["out"].reshape(128, 8, 2, 256)      # [p, ic, par, c]
        panels.append(o.transpose(1, 0, 2, 3).reshape(2048, 256))
    full = np.concatenate(panels, axis=1).astype(np.float32)
    return full[None, None], r


def kernel(X, Y):
    out, _ = _run(X, Y, trace=False)
    return out


# revision 39
# speedup vs baseline: 1.1201x; 1.1201x over previous
"""Jacobi 100-step solver on 8 trn2 cores via truncated DST-spectral transform.

v4: one explicit Jacobi step folded into mode space. With Qc the [N, K]
interior-DST basis (zero at boundary rows), the masked stencil satisfies
G1 = Qc^T x1 Qc = s_ab * G0 + 0.25*(rank-1 boundary spikes), G0 = Qc^T x0 Qc,
s_ab = lam_a + lam_b. The s_ab multiply rides the matmuls:
s_ab*G0 = (Qrows lam)^T B + Qrows^T (B lam), so the spikes share one PSUM
accumulation group and the G drain is a plain copy. x100 = Qc [W99*G1] Qc^T
on the (lo,lo)+(hi,hi) corners (K=256). Backward halves via Qhi = D Qlo:
out_odd = QloT_odd^T (Zlo+Zhi), out_even with (Zlo-Zhi), QloT parity-packed
on host. Forward hi-corner uses x0d = D*x0 against the same qlo panel.

Schedule notes (from neuron-profile traces): the 8 SPMD cores launch with
~30us skew and the first collective pays a barrier + setup latency, so a
zero-payload warmup AllReduce is doorbelled at ~11us (barrier_in DMA rides
first on the sync ring); the real AllReduce then chains ~2us after its
doorbell. All loads are single large DMA triggers striped over all 16 DMA
engines, ordered so forward-critical bytes land first; xy is quartered so
elementwise block 0 starts ~5us earlier; backward consts stream during the
AllReduce window. Spike vectors upload as a [128,16] compact blob and are
scattered to a [1, 2048] row by a tiny SBUF-to-SBUF DMA.
"""

import sys
import types
import numpy as np

N = 2048
NC = 8
P = N // NC          # 256 panel columns per core
K = 256              # spectral corner size per corner
RC = N // 128        # 16 row chunks

# qb blob (shared across cores): [qloA 2048 | ident 128 | lamr 1024 | qloB 2048]
QB_IDENT = 2048
QB_LAMR = 2176       # (jm, t) slot s=2jm+t -> lam_t[a] replicated over partitions
QB_B = 3200
QBW = 5248
# cbQ blob (per-core): [qrows 1024 | qrowsL 1024 | svc 16]
OF_QROWS = 0         # (jm, t) -> Qc_t[256*core + jm*128 + p, b]
OF_QROWSL = 1024     # same * lam_t[b]
OF_SVC = 2048        # [128, 16] compact: (p, j) = svflat[p*16 + j]
CBQW = 2064
# svflat row (scattered to svusv[0:1, 0:2048]):
# [0:1024]   svu: svec_lo | svecp_lo | svec_hi | svecp_hi       (x0.25)
# [1024:2048] svv: vleft lo | vleft hi | vright lo | vright hi  (x0.25, edge sel)
# cbB blob (backward consts): [qrowsT 1024 | w99 1024]
OF_QROWST = 0        # (t, bm) -> Qc_t[256*core + c, bm*128 + p]
OF_W99 = 1024        # (t, bm) -> W99_t[bm*128+p, a]
CBBW = 2048


def _install_ntff_hook():
    if "antenv.axon_hooks" in sys.modules:
        return
    mod = types.ModuleType("antenv.axon_hooks")
    mod._hook = None
    mod.set_axon_ntff_profile_hook = lambda h: setattr(mod, "_hook", h)
    mod.get_axon_ntff_profile_hook = lambda: mod._hook
    sys.modules["antenv.axon_hooks"] = mod
    try:
        import antenv
        antenv.axon_hooks = mod
        from trn_agent_boot.trn_boot import _ntff_profile_via_ctypes
        h = _ntff_profile_via_ctypes("/opt/axon/libaxon_pjrt.so")
        if h is not None:
            mod.set_axon_ntff_profile_hook(h)
    except Exception:
        pass


_HOST_CACHE = {}


def _host_constants():
    if _HOST_CACHE:
        return _HOST_CACHE
    i = np.arange(N, dtype=np.float64)
    Qs, lams, svecs, svecps = [], [], [], []
    for lo in (True, False):
        m = np.arange(1, K + 1, dtype=np.float64) if lo else np.arange(N - 2, N - 2 - K, -1, dtype=np.float64)
        red = np.outer(i, m) % (2 * (N - 1))
        Qc = np.sqrt(2.0 / (N - 1)) * np.sin(np.pi * red / (N - 1))   # [N, K]
        lam = 0.5 * np.cos(np.pi * m / (N - 1))
        Qs.append(Qc)
        lams.append(lam)
        svecs.append(Qc[1, :].copy())
        svecps.append(Qc[N - 2, :].copy())

    qb = np.zeros((128, QBW), np.float64)
    for r in range(RC):
        off = r * 256 if r < 8 else QB_B + (r - 8) * 256
        qb[:, off: off + 256] = Qs[0][128 * r:128 * (r + 1), :]
    qb[:, QB_IDENT: QB_IDENT + 128] = np.eye(128)
    for jm in range(2):
        for t in range(2):
            s = 2 * jm + t
            qb[:, QB_LAMR + s * 256: QB_LAMR + (s + 1) * 256] = lams[t][None, :]

    # qcT parity packed [128, 4096]: [am, par, ic, j] = Qlo[256*ic + 2*j + par, 128*am + p]
    qcT = np.zeros((128, 4096), np.float64)
    j = np.arange(128)
    for am in range(2):
        for par in range(2):
            for ic in range(8):
                rows = 256 * ic + 2 * j + par
                qcT[:, am * 2048 + par * 1024 + ic * 128: am * 2048 + par * 1024 + (ic + 1) * 128] = \
                    Qs[0][rows, 128 * am:128 * (am + 1)].T

    w99T = np.zeros((128, 1024), np.float64)
    for t in range(2):
        sab = lams[t][:, None] + lams[t][None, :]      # [b, a] (symmetric)
        w99 = sab ** 99
        for bm in range(2):
            s = 2 * t + bm
            w99T[:, s * 256:(s + 1) * 256] = w99[bm * 128:(bm + 1) * 128, :]

    _HOST_CACHE.update(qb=qb.astype(np.float16), qcT=qcT.astype(np.float16),
                       w99T=w99T, lams=lams, Qs=Qs, svecs=svecs, svecps=svecps)
    return _HOST_CACHE


def _core_cbQ(c, hc):
    Qs, svecs, svecps, lams = hc["Qs"], hc["svecs"], hc["svecps"], hc["lams"]
    cb = np.zeros((128, CBQW), np.float64)
    for jm in range(2):
        for t in range(2):
            s = 2 * jm + t
            rows = Qs[t][256 * c + 128 * jm: 256 * c + 128 * (jm + 1), :]
            cb[:, OF_QROWS + s * 256: OF_QROWS + (s + 1) * 256] = rows
            cb[:, OF_QROWSL + s * 256: OF_QROWSL + (s + 1) * 256] = rows * lams[t][None, :]
    svflat = np.zeros(2048, np.float64)
    for t in range(2):
        svflat[t * 512: t * 512 + 256] = 0.25 * svecs[t]
        svflat[t * 512 + 256: t * 512 + 512] = 0.25 * svecps[t]
    if c == 0:
        for t in range(2):
            svflat[1024 + t * 256: 1024 + (t + 1) * 256] = 0.25 * svecs[t]
    if c == NC - 1:
        for t in range(2):
            svflat[1536 + t * 256: 1536 + (t + 1) * 256] = 0.25 * svecps[t]
    cb[:, OF_SVC: OF_SVC + 16] = svflat.reshape(128, 16)
    return cb.astype(np.float16)


def _core_cbB(c, hc):
    Qs = hc["Qs"]
    cb = np.zeros((128, CBBW), np.float64)
    for t in range(2):
        for bm in range(2):
            s = 2 * t + bm
            cb[:, OF_QROWST + s * 256: OF_QROWST + (s + 1) * 256] = \
                Qs[t][256 * c: 256 * (c + 1), 128 * bm: 128 * (bm + 1)].T
    cb[:, OF_W99: OF_W99 + 1024] = hc["w99T"]
    return cb.astype(np.float16)


_NC_CACHE = {}


def _build():
    if "nc" in _NC_CACHE:
        return _NC_CACHE["nc"]
    import concourse.bacc as bacc
    import concourse.tile as tile
    import concourse.mybir as mybir

    F16 = mybir.dt.float16
    F32 = mybir.dt.float32
    ALU = mybir.AluOpType
    ACTF = mybir.ActivationFunctionType
    nc = bacc.Bacc("TRN2", target_bir_lowering=False, debug=False, num_devices=NC)

    # xyb col 0 = sgncol ((-1)^(p+1)); then 4 blocks of [x 1024 | y 1024]
    xyb_d = nc.dram_tensor("xyb", [128, 8193], F16, kind="ExternalInput")
    qb_d = nc.dram_tensor("qb", [128, QBW], F16, kind="ExternalInput")
    cbQ_d = nc.dram_tensor("cbQ", [128, CBQW], F16, kind="ExternalInput")
    cbB_d = nc.dram_tensor("cbB", [128, CBBW], F16, kind="ExternalInput")
    qcT_d = nc.dram_tensor("qcT", [128, 4096], F16, kind="ExternalInput")
    out_d = nc.dram_tensor("out", [128, 4096], F16, kind="ExternalOutput")

    with tile.TileContext(nc) as tc:
        with tc.tile_pool(name="pers", bufs=1) as pers, \
             tc.tile_pool(name="ps", bufs=1, space="PSUM") as ps, \
             tc.tile_pool(name="dram", bufs=1, space="DRAM") as dram:

            # ---- persistent SBUF ----
            xyb_s = pers.tile([128, 8193], F16, tag="xyb")
            qb_s = pers.tile([128, QBW], F16, tag="qb")
            cbQ_s = pers.tile([128, CBQW], F16, tag="cbQ")
            cbB_s = pers.tile([128, CBBW], F16, tag="cbB")
            qcT_s = pers.tile([128, 4096], F16, tag="qcT")
            x0b = pers.tile([128, 4096], F16, tag="x0b")
            x0d = pers.tile([128, 4096], F16, tag="x0d")
            t2b = pers.tile([128, 4096], F32, tag="t2b")
            t3b = pers.tile([128, 4096], F32, tag="t3b")
            d2b = pers.tile([128, 4096], F32, tag="d2b")
            abuf = pers.tile([128, 1024], F16, tag="abuf")
            abufL = pers.tile([128, 1024], F16, tag="abufL")
            usb = pers.tile([128, 1024], F16, tag="usb")
            svusv = pers.tile([128, 2048], F16, tag="svusv")
            x0rT = pers.tile([128, 4], F16, tag="x0rT")
            vrow = pers.tile([128, 512], F16, tag="vrow")
            gsb = pers.tile([128, 1024], F16, tag="gsb")
            gout_s = pers.tile([128, 1024], F16, tag="gouts")
            utb = pers.tile([128, 1024], F16, tag="utb")
            zbuf = pers.tile([128, 1024], F16, tag="zbuf")
            ztmp = pers.tile([128, 512], F16, tag="ztmp")
            outb = pers.tile([128, 4096], F16, tag="outb")
            zt = pers.tile([128, 1], F32, tag="zt")

            # const APs for activation biases
            cexp = pers.tile([128, 1], F32, tag="cexp", name="cexp")
            nc.vector.memset(cexp[:], -12.5)
            nc.const_aps.aps[(F32, -12.5)] = cexp[:]
            csq = pers.tile([128, 1], F32, tag="csq", name="csq")
            nc.vector.memset(csq[:], -0.5)
            nc.const_aps.aps[(F32, -0.5)] = csq[:]

            # ---- warmup AllReduce: barrier_in rides FIRST on the sync ring so
            # the gpsimd doorbell rings at ~11us and the NEFF collective
            # barrier + first-op setup burn during forward ----
            barrier_in = dram.tile([128, 1], F32, tag="barin")
            barrier_out = dram.tile([128, 1], F32, tag="barout", addr_space="Shared")
            nc.vector.memset(zt[:], 0.0)
            nc.sync.dma_start(barrier_in[:, :], zt[:])
            nc.gpsimd.collective_compute(
                "AllReduce", ALU.add,
                replica_groups=[list(range(NC))],
                ins=[barrier_in.opt()], outs=[barrier_out.opt()],
            )

            # ---- loads: strict FIFO order on the sync ring; xy quartered ----
            nc.sync.dma_start(xyb_s[:, 0:2049], xyb_d[:, 0:2049])        # sgn + blk0
            nc.sync.dma_start(xyb_s[:, 2049:4097], xyb_d[:, 2049:4097])  # blk1
            nc.sync.dma_start(qb_s[:, 0:QB_B], qb_d[:, 0:QB_B])          # qloA+ident+lamr
            nc.sync.dma_start(xyb_s[:, 4097:6145], xyb_d[:, 4097:6145])  # blk2
            nc.sync.dma_start(xyb_s[:, 6145:8193], xyb_d[:, 6145:8193])  # blk3
            nc.sync.dma_start(qb_s[:, QB_B:QBW], qb_d[:, QB_B:QBW])      # qloB
            nc.sync.dma_start(cbQ_s[:], cbQ_d[:, :])
            nc.sync.dma_start(cbB_s[:], cbB_d[:, :])
            nc.sync.dma_start(qcT_s[:], qcT_d[:, :])
            # scatter compact spike vectors to a [1, 2048] row (partition 0);
            # on the sync ring so its wait-for-cbQ doesn't stall the scalar queue
            nc.sync.dma_start(svusv[0:1, 0:2048], cbQ_s[:, OF_SVC:OF_SVC + 16])

            sgncol = xyb_s[:, 0:1]
            ident_s = qb_s[:, QB_IDENT:QB_IDENT + 128]

            # ---- A accumulators: [c-part(jm), modes] x (lo, hi) ----
            aps = [ps.tile([128, K], F32, tag="aacc", bufs=4, name=f"aps{j}") for j in range(4)]
            # slot j = 2*jm + t

            def qslot(r):
                off = r * 256 if r < 8 else QB_B + (r - 8) * 256
                return qb_s[:, off: off + 256]

            def mm1_chunk(r):
                for jm in range(2):
                    sl = slice(r * 256 + jm * 128, r * 256 + (jm + 1) * 128)
                    nc.tensor.matmul(aps[2 * jm][:], x0b[:, sl], qslot(r),
                                     start=(r == 0), stop=(r == RC - 1))
                for jm in range(2):
                    sl = slice(r * 256 + jm * 128, r * 256 + (jm + 1) * 128)
                    nc.tensor.matmul(aps[2 * jm + 1][:], x0d[:, sl], qslot(r),
                                     start=(r == 0), stop=(r == RC - 1))

            def fwd_block(blk):
                sl = slice(blk * 1024, (blk + 1) * 1024)
                xs = xyb_s[:, 1 + blk * 2048: 1 + blk * 2048 + 1024]
                ys = xyb_s[:, 1 + blk * 2048 + 1024: 1 + (blk + 1) * 2048]
                # d2' = X(X-1) + (Y-.5)^2 = d^2 - 0.25
                nc.vector.scalar_tensor_tensor(t2b[:, sl], xs, -1.0, xs, ALU.add, ALU.mult)
                nc.scalar.activation(t3b[:, sl], ys, ACTF.Square, bias=-0.5, scale=1.0)
                nc.gpsimd.tensor_add(d2b[:, sl], t2b[:, sl], t3b[:, sl])
                # x0 = exp(-50*d^2) = exp(-50*d2' - 12.5)
                nc.scalar.activation(x0b[:, sl], d2b[:, sl], ACTF.Exp, bias=-12.5, scale=-50.0)
                # hi-corner operand: x0d = D x0 (checkerboard row sign)
                nc.vector.tensor_mul(x0d[:, sl], x0b[:, sl], sgncol.to_broadcast((128, 1024)))
                for r in range(4 * blk, 4 * blk + 4):
                    mm1_chunk(r)

            def row_transpose(col_lo, out_col, take_row):
                tp = ps.tile([128, 128], F16, tag="pp", bufs=4, name="pp")
                nc.tensor.transpose(tp[:], x0b[:, col_lo:col_lo + 128], ident_s)
                nc.vector.tensor_copy(x0rT[:, out_col:out_col + 1], tp[:, take_row:take_row + 1])

            def u_project(xcol0, xcol1, dst_off):
                for t in range(2):
                    ups = ps.tile([128, K], F32, tag="pp", bufs=4, name="pp")
                    nc.tensor.matmul(ups[0:1, :], x0rT[:, xcol0:xcol0 + 1],
                                     cbQ_s[:, OF_QROWS + t * 256: OF_QROWS + (t + 1) * 256],
                                     start=True, stop=False)
                    nc.tensor.matmul(ups[0:1, :], x0rT[:, xcol1:xcol1 + 1],
                                     cbQ_s[:, OF_QROWS + (2 + t) * 256: OF_QROWS + (3 + t) * 256],
                                     start=False, stop=True)
                    nc.vector.tensor_copy(usb[0:1, dst_off + t * 256: dst_off + (t + 1) * 256], ups[0:1, :])

            fwd_block(0)
            fwd_block(1)
            # top boundary row machinery early (ident lands with qbA)
            row_transpose(0, 0, 0)
            row_transpose(128, 1, 0)
            fwd_block(2)
            fwd_block(3)
            row_transpose(15 * 256, 2, 127)
            row_transpose(15 * 256 + 128, 3, 127)
            u_project(0, 1, 0)                     # usb[0:512] = u_top (lo|hi)
            u_project(2, 3, 512)                   # usb[512:1024] = u_bot (lo|hi)

            # ---- drain A -> abuf and abufL = A * lam_a ----
            for j in range(4):
                jm, t = j // 2, j % 2
                sl = slice(jm * 512 + t * 256, jm * 512 + (t + 1) * 256)
                nc.scalar.copy(abuf[:, sl], aps[j][:])
                nc.vector.tensor_mul(abufL[:, sl], aps[j][:], qb_s[:, QB_LAMR + j * 256: QB_LAMR + (j + 1) * 256])

            # v_right row (panel col 255) lives at abuf partition 127; PE operands
            # must start at partition 0 -> stage it down via SBUF-to-SBUF DMA
            nc.sync.dma_start(vrow[0:1, :], abuf[127:128, 512:1024])

            # ---- mm2: G1^T = (Qrows L)^T B + Qrows^T (B L) + rank-1 boundary
            # spikes, one psum accumulation group per (t, bm) ----
            gin = dram.tile([128, 1024], F16, tag="gin")
            gout = dram.tile([128, 1024], F16, tag="gout", addr_space="Shared")
            for t in range(2):
                for bm in range(2):
                    s = 2 * t + bm
                    gp = ps.tile([128, K], F32, tag="pp", bufs=4, name="pp")
                    for jm in range(2):
                        qsl = slice(OF_QROWS + (2 * jm + t) * 256 + bm * 128,
                                    OF_QROWS + (2 * jm + t) * 256 + (bm + 1) * 128)
                        qLsl = slice(OF_QROWSL + (2 * jm + t) * 256 + bm * 128,
                                     OF_QROWSL + (2 * jm + t) * 256 + (bm + 1) * 128)
                        asl = slice(jm * 512 + t * 256, jm * 512 + (t + 1) * 256)
                        nc.tensor.matmul(gp[:], cbQ_s[:, qLsl], abuf[:, asl],
                                         start=(jm == 0), stop=False)
                        nc.tensor.matmul(gp[:], cbQ_s[:, qsl], abufL[:, asl],
                                         start=False, stop=False)
                    nc.tensor.matmul(gp[:], usb[0:1, t * 256 + bm * 128: t * 256 + (bm + 1) * 128],
                                     svusv[0:1, t * 512: t * 512 + 256],
                                     start=False, stop=False)
                    nc.tensor.matmul(gp[:], usb[0:1, 512 + t * 256 + bm * 128: 512 + t * 256 + (bm + 1) * 128],
                                     svusv[0:1, t * 512 + 256: t * 512 + 512],
                                     start=False, stop=False)
                    nc.tensor.matmul(gp[:], svusv[0:1, 1024 + t * 256 + bm * 128: 1024 + t * 256 + (bm + 1) * 128],
                                     abuf[0:1, t * 256: (t + 1) * 256],
                                     start=False, stop=False)
                    nc.tensor.matmul(gp[:], svusv[0:1, 1536 + t * 256 + bm * 128: 1536 + t * 256 + (bm + 1) * 128],
                                     vrow[0:1, t * 256: (t + 1) * 256],
                                     start=False, stop=True)
                    if s % 2 == 0:
                        nc.vector.tensor_copy(gsb[:, s * 256:(s + 1) * 256], gp[:])
                    else:
                        nc.scalar.copy(gsb[:, s * 256:(s + 1) * 256], gp[:])
            nc.sync.dma_start(gin[:, :], gsb[:])
            nc.gpsimd.collective_compute(
                "AllReduce", ALU.add,
                replica_groups=[list(range(NC))],
                ins=[gin.opt()], outs=[gout.opt()],
            )
            nc.sync.dma_start(gout_s[:], gout[:, :])

            # ---- filter: U^T = W99 * G1^T (SBUF-only) ----
            engs = (nc.vector, nc.gpsimd)
            for s in range(4):
                sl = slice(s * 256, (s + 1) * 256)
                engs[s % 2].tensor_mul(utb[:, sl], gout_s[:, sl],
                                       cbB_s[:, OF_W99 + s * 256: OF_W99 + (s + 1) * 256])

            # ---- B1: Z_t[a, c] = sum_b U_t[b, a] Qrows_t[c, b] ----
            # Zp (odd rows) at zbuf[0:512], Zm (even rows) at zbuf[512:1024]
            for am in range(2):
                zps = []
                for t in range(2):
                    zp = ps.tile([128, K], F32, tag="pp", bufs=4, name="pp")
                    for bm in range(2):
                        s = 2 * t + bm
                        nc.tensor.matmul(zp[:],
                                         utb[:, s * 256 + am * 128: s * 256 + (am + 1) * 128],
                                         cbB_s[:, OF_QROWST + s * 256: OF_QROWST + (s + 1) * 256],
                                         start=(bm == 0), stop=(bm == 1))
                    zps.append(zp)
                zsl = slice(am * 256, (am + 1) * 256)
                nc.scalar.copy(ztmp[:, zsl], zps[0][:])
                nc.vector.tensor_add(zbuf[:, am * 256: (am + 1) * 256], ztmp[:, zsl], zps[1][:])
                nc.vector.tensor_sub(zbuf[:, 512 + am * 256: 512 + (am + 1) * 256], ztmp[:, zsl], zps[1][:])

            # ---- B2: out chunks via parity-packed QloT; staged output DMA on
            # the idle sync ring (keeps the scalar queue free for drains) ----
            dr_engs = (nc.scalar, nc.vector)
            di = 0
            for ic in range(8):
                for par in range(2):
                    ops = ps.tile([128, K], F32, tag="pp", bufs=4, name="pp")
                    zoff = 0 if par == 1 else 512
                    for am in range(2):
                        nc.tensor.matmul(ops[:],
                                         qcT_s[:, am * 2048 + par * 1024 + ic * 128: am * 2048 + par * 1024 + (ic + 1) * 128],
                                         zbuf[:, zoff + am * 256: zoff + (am + 1) * 256],
                                         start=(am == 0), stop=(am == 1))
                    dst = outb[:, (2 * ic + par) * 256: (2 * ic + par + 1) * 256]
                    eng = dr_engs[di % 2]; di += 1
                    if eng is nc.scalar:
                        eng.copy(dst, ops[:])
                    else:
                        eng.tensor_copy(dst, ops[:])
                if ic == 3:
                    nc.sync.dma_start(out_d[:, 0:2048], outb[:, 0:2048])
                elif ic == 5:
                    nc.sync.dma_start(out_d[:, 2048:3072], outb[:, 2048:3072])
            nc.sync.dma_start(out_d[:, 3072:4096], outb[:, 3072:4096])

    nc.compile()
    _NC_CACHE["nc"] = nc
    return nc


def _run(X, Y, trace=False):
    _install_ntff_hook()
    from concourse.bass_utils import run_bass_kernel_spmd

    hc = _host_constants()
    Xf = np.asarray(X, np.float32).astype(np.float16)
    Yf = np.asarray(Y, np.float32).astype(np.float16)
    sgn = np.where(np.arange(128) % 2 == 1, 1.0, -1.0).astype(np.float16)

    in_maps = []
    for c in range(NC):
        xp = Xf[:, P * c: P * (c + 1)]           # [2048, 256]
        yp = Yf[:, P * c: P * (c + 1)]
        xyb = np.zeros((128, 8193), np.float16)
        xyb[:, 0] = sgn
        for r in range(RC):
            b, rr = r // 4, r % 4
            xyb[:, 1 + b * 2048 + rr * 256: 1 + b * 2048 + (rr + 1) * 256] = xp[128 * r:128 * (r + 1), :]
            xyb[:, 1 + b * 2048 + 1024 + rr * 256: 1 + b * 2048 + 1024 + (rr + 1) * 256] = yp[128 * r:128 * (r + 1), :]
        m = {"xyb": xyb,
             "qb": hc["qb"],
             "cbQ": _core_cbQ(c, hc),
             "cbB": _core_cbB(c, hc),
             "qcT": hc["qcT"]}
        in_maps.append(m)

    nc = _build()
    r = run_bass_kernel_spmd(nc, in_maps, core_ids=list(range(NC)), trace=trace)
    panels = []
    for c in range(NC):
        o = r.results[c]["out"].reshape(128, 8, 2, 256)      # [p, ic, par, c]
        panels.append(o.transpose(1, 0, 2, 3).reshape(2048, 256))
    full = np.concatenate(panels, axis=1).astype(np.float32)
    return full[None, None], r


def kernel(X, Y):
    out, _ = _run(X, Y, trace=False)
    return out
